# revision 10
# baseline (speedup 1.0000x reference)
"""Trainium2 Bass kernel for a 2-layer spiking LSTM (SLSTM) with temporal
attenuation readout.

Model (per timestep t, per batch row b):
    gates1 = x_t @ W_ih1.T + b_ih1 + mem1 @ W_hh1.T + b_hh1
    i,f,g,o = split(gates1); c1 = sig(f)*c1 + sig(i)*tanh(g); h1 = sig(o)*tanh(c1)
    mem1 = h1 - thr1*(mem1_prev > thr1);  spk1 = (mem1 > thr1)
    ... same for layer 2 with input spk1 ...
    out = (sum_t w_t * mem2_t) @ W_fc.T + b_fc,  w_t = exp(-a*(T-1-t))/Z

Sharding: data-parallel over batch B=256 across 8 cores (32 rows each);
weights replicated; the T=512 recurrence runs locally per core.

Per-core layouts (B_c = 32 batch rows/core, H = 512 = 4 chunks of 128):
  P-layout (states):  SBUF tile [128, 128], partition = X*32+b (X = h-chunk,
                      b = batch), free = hh (h within chunk); h = X*128+hh.
  Gates PSUM tile [128, 512]: partition = (X, b), free = (gate, hh) with
                      gate order (i, f, o, g).
  T-layout (matmul stationary): transpose of P-layout: [128 = hh, 128 =
                      (X, b)]; contraction chunk kc = T[:, kc*32:(kc+1)*32].

Matmuls put the (transposed) recurrent state as the 128x32 stationary
operand, col-tiled 4x across the PE array (tile_position=(0, 32*X)); each
col-strip streams its own 512-wide slice of the host-prepacked weights, so
the whole array is busy and gates land directly in elementwise-friendly
layout. Biases ride in via an appended ones-row (layer 1: augmented x row;
layer 2: K=1 ones matmul). The temporal weighting uses a Horner recurrence
S = decay*S + mem2 with the normalization folded into W_fc.
"""

import sys

if "/opt/trn_rl_repo" not in sys.path:
    sys.path.insert(0, "/opt/trn_rl_repo")

import numpy as np

import concourse.mybir as mybir
from concourse import bacc
from concourse.bass_utils import run_bass_kernel_spmd
from concourse.tile import TileContext

F32 = mybir.dt.float32
ALU = mybir.AluOpType
AFT = mybir.ActivationFunctionType

ALPHA = 0.05
N_CORES = 8
H = 512
I_IN = 14
NCH = H // 128  # 4 h-chunks
BC = 32  # batch per core
# our gate order (i, f, o, g) expressed as indices into the original
# (i, f, g, o) row blocks of the torch-style weight matrices
GATE_PERM = [0, 1, 3, 2]


def _prep_rec_weight(W: np.ndarray) -> np.ndarray:
    """W [4H, K] -> rhs pack [128, (kc, X, gate, hh)] with K = nkc*128."""
    K = W.shape[1]
    nkc = K // 128
    W4 = W.reshape(4, H, K)[GATE_PERM]  # [gate, h_out, k]
    W4 = W4.reshape(4, NCH, 128, nkc, 128)  # [gate, X, hh, kc, kk]
    W4 = W4.transpose(4, 3, 1, 0, 2)  # [kk, kc, X, gate, hh]
    return np.ascontiguousarray(W4.reshape(128, nkc * 4 * 4 * 128), np.float32)


def _prep_ih1(W_ih1: np.ndarray, bias1: np.ndarray) -> np.ndarray:
    """[4H, 14] + bias [4H] -> [15, (X, gate, hh)]."""
    Wa = np.concatenate([W_ih1, bias1[:, None]], axis=1)  # [4H, 15]
    W4 = Wa.reshape(4, H, 15)[GATE_PERM].reshape(4, NCH, 128, 15)
    W4 = W4.transpose(3, 1, 0, 2)  # [k, X, gate, hh]
    return np.ascontiguousarray(W4.reshape(15, 4 * 4 * 128), np.float32)


def _prep_bias_row(bias: np.ndarray) -> np.ndarray:
    """bias [4H] -> [1, (X, gate, hh)]."""
    b4 = bias.reshape(4, H)[GATE_PERM].reshape(4, NCH, 128)
    b4 = b4.transpose(1, 0, 2)  # [X, gate, hh]
    return np.ascontiguousarray(b4.reshape(1, 4 * 4 * 128), np.float32)


def build_program(T: int, thr1: float, thr2: float):
    """Emit the full Bass/Tile program for one core (SPMD across 8)."""
    nc = bacc.Bacc("TRN2", target_bir_lowering=False, debug=False,
                   num_devices=N_CORES)

    x_d = nc.declare_dram_parameter("x_aug", [15, T * BC], F32, isOutput=False)
    wih1_d = nc.declare_dram_parameter("wih1", [15, 4 * 4 * 128], F32, isOutput=False)
    whh1_d = nc.declare_dram_parameter("whh1", [128, NCH * 2048], F32, isOutput=False)
    wih2_d = nc.declare_dram_parameter("wih2", [128, NCH * 2048], F32, isOutput=False)
    whh2_d = nc.declare_dram_parameter("whh2", [128, NCH * 2048], F32, isOutput=False)
    b2_d = nc.declare_dram_parameter("b2row", [1, 4 * 4 * 128], F32, isOutput=False)
    wfc_d = nc.declare_dram_parameter("wfc", [128, NCH * 8], F32, isOutput=False)
    bfc_d = nc.declare_dram_parameter("bfc", [1, 8], F32, isOutput=False)
    id_d = nc.declare_dram_parameter("ident", [128, 128], F32, isOutput=False)
    out_d = nc.declare_dram_parameter("out", [BC, 8], F32, isOutput=True)

    decay = float(np.float32(np.exp(np.float32(-ALPHA))))

    with TileContext(nc) as tc:
        with (
            tc.tile_pool(name="const", bufs=1) as cpool,
            tc.tile_pool(name="state", bufs=1) as spool,
            tc.tile_pool(name="work", bufs=1) as wpool,
            tc.tile_pool(name="xin", bufs=8) as xpool,
            tc.tile_pool(name="ps1", bufs=2, space="PSUM") as ps1pool,
            tc.tile_pool(name="ps2", bufs=2, space="PSUM") as ps2pool,
            tc.tile_pool(name="psx", bufs=2, space="PSUM") as psxpool,
            tc.tile_pool(name="psfc", bufs=1, space="PSUM") as psfcpool,
        ):
            # ---- constants into SBUF ----
            wih1 = cpool.tile([15, 4 * 4 * 128], F32)
            nc.sync.dma_start(out=wih1[:, :], in_=wih1_d[:, :])
            whh1 = cpool.tile([128, NCH * 2048], F32)
            nc.sync.dma_start(out=whh1[:, :], in_=whh1_d[:, :])
            wih2 = cpool.tile([128, NCH * 2048], F32)
            nc.sync.dma_start(out=wih2[:, :], in_=wih2_d[:, :])
            whh2 = cpool.tile([128, NCH * 2048], F32)
            nc.sync.dma_start(out=whh2[:, :], in_=whh2_d[:, :])
            b2row = cpool.tile([1, 4 * 4 * 128], F32)
            nc.sync.dma_start(out=b2row[:, :], in_=b2_d[:, :])
            wfc = cpool.tile([128, NCH * 8], F32)
            nc.sync.dma_start(out=wfc[:, :], in_=wfc_d[:, :])
            bfc = cpool.tile([1, 8], F32)
            nc.sync.dma_start(out=bfc[:, :], in_=bfc_d[:, :])
            ident = cpool.tile([128, 128], F32)
            nc.sync.dma_start(out=ident[:, :], in_=id_d[:, :])
            ones = cpool.tile([1, BC], F32)
            nc.gpsimd.memset(ones[:, :], 1.0)

            # ---- persistent state tiles (P-layout [128, 128]) ----
            mem1 = spool.tile([128, 128], F32)
            mem2 = spool.tile([128, 128], F32)
            mem1T = spool.tile([128, 128], F32)
            mem2T = spool.tile([128, 128], F32)
            spk1P = spool.tile([128, 128], F32)
            spk1T = spool.tile([128, 128], F32)
            spk2P = spool.tile([128, 128], F32)
            acc = [spool.tile([128, 128], F32, name="accA"),
                   spool.tile([128, 128], F32, name="accB")]
            gc1 = spool.tile([128, 256], F32)  # [tanh(g) | c1]
            gc2 = spool.tile([128, 256], F32)
            for t_ in (mem1, mem2, mem1T, mem2T, spk1P, spk1T, spk2P,
                       acc[0], acc[1], gc1, gc2):
                nc.gpsimd.memset(t_[:, :], 0.0)

            # ---- per-step work tiles ----
            sig1 = wpool.tile([128, 384], F32)
            sig2 = wpool.tile([128, 384], F32)
            prod1 = wpool.tile([128, 256], F32)
            prod2 = wpool.tile([128, 256], F32)
            tc1 = wpool.tile([128, 128], F32)
            tc2 = wpool.tile([128, 128], F32)
            h1 = wpool.tile([128, 128], F32)
            h2 = wpool.tile([128, 128], F32)

            xp2_prev = None  # psum tile holding transpose(mem2) of prev step

            def mm(ps, lhsT, rhs, X, start, stop):
                nc.tensor.matmul(
                    ps[32 * X:32 * (X + 1), :],
                    lhsT,
                    rhs,
                    start=start,
                    stop=stop,
                    tile_position=(0, 32 * X),
                    skip_group_check=True,
                )

            for t in range(T):
                # ---------------- layer 1 matmuls -> ps1 ----------------
                xt = xpool.tile([15, BC], F32, tag="xt")
                nc.sync.dma_start(out=xt[:, :], in_=x_d[:, t * BC:(t + 1) * BC])
                ps1 = ps1pool.tile([128, 512], F32, tag="ps1")
                xs = xt[:, :]
                for X in range(4):
                    mm(ps1, xs, wih1[:, X * 512:(X + 1) * 512], X, True, False)
                for kc in range(NCH):
                    st = mem1T[:, kc * 32:(kc + 1) * 32]
                    for X in range(4):
                        mm(ps1, st,
                           whh1[:, kc * 2048 + X * 512: kc * 2048 + (X + 1) * 512],
                           X, False, kc == NCH - 1)

                # transpose(mem2) of previous step, then its SBUF copy
                if t > 0:
                    nc.tensor.transpose(xp2_prev[:, :], mem2[:, :], ident[:, :])
                    nc.vector.tensor_copy(mem2T[:, :], xp2_prev[:, :])

                # ---------------- layer 2 recurrent matmuls -> ps2 ------
                ps2 = ps2pool.tile([128, 512], F32, tag="ps2")
                for kc in range(NCH):
                    st = mem2T[:, kc * 32:(kc + 1) * 32]
                    for X in range(4):
                        mm(ps2, st,
                           whh2[:, kc * 2048 + X * 512: kc * 2048 + (X + 1) * 512],
                           X, kc == 0, False)

                # ---------------- layer 1 elementwise -------------------
                nc.scalar.activation(sig1[:, :], ps1[:, 0:384], AFT.Sigmoid)
                nc.scalar.activation(gc1[:, 0:128], ps1[:, 384:512], AFT.Tanh)
                nc.vector.tensor_mul(prod1[:, :], sig1[:, 0:256], gc1[:, 0:256])
                nc.vector.tensor_add(gc1[:, 128:256], prod1[:, 0:128],
                                     prod1[:, 128:256])
                nc.scalar.activation(tc1[:, :], gc1[:, 128:256], AFT.Tanh)
                nc.vector.tensor_mul(h1[:, :], sig1[:, 256:384], tc1[:, :])
                # mem1 = h1 - thr1*reset ; reset = spk of previous step
                nc.vector.scalar_tensor_tensor(
                    mem1[:, :], spk1P[:, :], -thr1, h1[:, :], ALU.mult, ALU.add)
                nc.vector.tensor_scalar(
                    spk1P[:, :], mem1[:, :], thr1, None, ALU.is_gt)

                # transpose mem1 -> T-layout; spike in T-layout feeds L2
                xp1 = psxpool.tile([128, 128], F32, tag="xp")
                nc.tensor.transpose(xp1[:, :], mem1[:, :], ident[:, :])
                nc.vector.tensor_copy(mem1T[:, :], xp1[:, :])
                nc.vector.tensor_scalar(
                    spk1T[:, :], mem1T[:, :], thr1, None, ALU.is_gt)

                # ---------------- layer 2 input matmuls + bias ----------
                for kc in range(NCH):
                    st = spk1T[:, kc * 32:(kc + 1) * 32]
                    for X in range(4):
                        mm(ps2, st,
                           wih2[:, kc * 2048 + X * 512: kc * 2048 + (X + 1) * 512],
                           X, False, False)
                for X in range(4):
                    mm(ps2, ones[:, :], b2row[:, X * 512:(X + 1) * 512],
                       X, False, True)

                # ---------------- layer 2 elementwise -------------------
                nc.scalar.activation(sig2[:, :], ps2[:, 0:384], AFT.Sigmoid)
                nc.scalar.activation(gc2[:, 0:128], ps2[:, 384:512], AFT.Tanh)
                nc.vector.tensor_mul(prod2[:, :], sig2[:, 0:256], gc2[:, 0:256])
                nc.vector.tensor_add(gc2[:, 128:256], prod2[:, 0:128],
                                     prod2[:, 128:256])
                nc.scalar.activation(tc2[:, :], gc2[:, 128:256], AFT.Tanh)
                nc.vector.tensor_mul(h2[:, :], sig2[:, 256:384], tc2[:, :])
                nc.vector.scalar_tensor_tensor(
                    mem2[:, :], spk2P[:, :], -thr2, h2[:, :], ALU.mult, ALU.add)
                nc.vector.tensor_scalar(
                    spk2P[:, :], mem2[:, :], thr2, None, ALU.is_gt)
                # temporal attenuation (Horner): acc = decay*acc + mem2
                nc.vector.scalar_tensor_tensor(
                    acc[(t + 1) % 2][:, :], acc[t % 2][:, :], decay,
                    mem2[:, :], ALU.mult, ALU.add)

                if t < T - 1:
                    xp2_prev = psxpool.tile([128, 128], F32, tag="xp")

            # ---------------- readout: out = accT-weighted FC ----------
            acc_fin = acc[T % 2]
            xpa = psxpool.tile([128, 128], F32, tag="xp")
            nc.tensor.transpose(xpa[:, :], acc_fin[:, :], ident[:, :])
            accT = wpool.tile([128, 128], F32)
            nc.vector.tensor_copy(accT[:, :], xpa[:, :])
            psfc = psfcpool.tile([32, 8], F32, tag="fc")
            for kc in range(NCH):
                nc.tensor.matmul(
                    psfc[:, :], accT[:, kc * 32:(kc + 1) * 32],
                    wfc[:, kc * 8:(kc + 1) * 8],
                    start=(kc == 0), stop=False, skip_group_check=True)
            nc.tensor.matmul(psfc[:, :], ones[:, :], bfc[:, :],
                             start=False, stop=True, skip_group_check=True)
            outsb = wpool.tile([32, 8], F32)
            nc.vector.tensor_copy(outsb[:, :], psfc[:, :])
            nc.sync.dma_start(out=out_d[:, :], in_=outsb[:, :])

    nc.compile()
    return nc


def prep_inputs(x, W_ih1, W_hh1, b_ih1, b_hh1, W_ih2, W_hh2, b_ih2, b_hh2,
                W_fc, b_fc, T):
    """Host-side packing into per-core in_maps."""
    x = np.asarray(x, np.float32)
    # normalization constant of the attenuation weights (folded into W_fc)
    w32 = np.exp(np.float32(-ALPHA) * np.arange(T - 1, -1, -1, dtype=np.float32))
    Z = float(np.float64(w32.sum()))

    wih1 = _prep_ih1(np.asarray(W_ih1, np.float32),
                     np.asarray(b_ih1, np.float32) + np.asarray(b_hh1, np.float32))
    whh1 = _prep_rec_weight(np.asarray(W_hh1, np.float32))
    wih2 = _prep_rec_weight(np.asarray(W_ih2, np.float32))
    whh2 = _prep_rec_weight(np.asarray(W_hh2, np.float32))
    b2 = _prep_bias_row(np.asarray(b_ih2, np.float32) + np.asarray(b_hh2, np.float32))
    # wfc [128, (kc, c)]: wfc[kk, kc*8+c] = W_fc[c, kc*128+kk] / Z
    wfc = (np.asarray(W_fc, np.float64) / Z).astype(np.float32)  # [8, 512]
    wfc = wfc.reshape(8, NCH, 128).transpose(2, 1, 0)  # [kk, kc, c]
    wfc = np.ascontiguousarray(wfc.reshape(128, NCH * 8), np.float32)
    bfc = np.asarray(b_fc, np.float32).reshape(1, 8)
    ident = np.eye(128, dtype=np.float32)

    in_maps = []
    for c in range(N_CORES):
        xs = x[:, c * BC:(c + 1) * BC, :]  # [T, 32, 14]
        x_aug = np.empty((15, T * BC), np.float32)
        x_aug[:14] = xs.transpose(2, 0, 1).reshape(14, T * BC)
        x_aug[14] = 1.0
        in_maps.append({
            "x_aug": x_aug, "wih1": wih1, "whh1": whh1, "wih2": wih2,
            "whh2": whh2, "b2row": b2, "wfc": wfc, "bfc": bfc, "ident": ident,
        })
    return in_maps


_CACHE = {}


def run(trace=False, **inputs):
    """Build+run; returns (out [B, 8] float32, BassKernelResults)."""
    x = np.asarray(inputs["x"], np.float32)
    T = x.shape[0]
    thr1 = float(np.asarray(inputs["thr1"]))
    thr2 = float(np.asarray(inputs["thr2"]))
    key = (T, thr1, thr2)
    if key not in _CACHE:
        _CACHE[key] = build_program(T, thr1, thr2)
    nc = _CACHE[key]
    in_maps = prep_inputs(
        x, inputs["W_ih1"], inputs["W_hh1"], inputs["b_ih1"], inputs["b_hh1"],
        inputs["W_ih2"], inputs["W_hh2"], inputs["b_ih2"], inputs["b_hh2"],
        inputs["W_fc"], inputs["b_fc"], T)
    res = run_bass_kernel_spmd(nc, in_maps, core_ids=list(range(N_CORES)),
                               trace=trace)
    out = np.concatenate([r["out"] for r in res.results], axis=0)
    return np.ascontiguousarray(out, np.float32), res


def kernel(**inputs) -> np.ndarray:
    out, _ = run(trace=False, **inputs)
    return out
